# revision 1
# baseline (speedup 1.0000x reference)
"""MoE with adaptive gate on 8 trn2 NeuronCores.

Strategy: data-parallel over the batch. Each core gets B/8 = 1024 rows of x
(pre-transposed on host to [D, Bs]) plus a full replica of all expert
weights, and computes its slice of the output entirely locally — no
collectives. On-chip layout is feature-major ([feature, batch]) end to end:

  gates:  psum[e, b]  = sum_dc gw[dc].T @ xT[dc]         (K=128 x 16, N=512)
          softmax over e via Exp + ones-matmul partition reduction
  L1:     psum[h, b]  = sum_dc W1[e,dc].T @ xT[dc]       -> silu -> h1
  L2:     psum[k, b]  = W2[e].T @ h1                     -> silu -> h2
  gate:   h2s[e]      = h2 * bcast(gates[e])             (one-hot matmul bcast)
  L3:     psum[d, b]  = sum_e W3[e,dc].T @ h2s[e]        (all experts in PSUM)

All matmuls run as float32r (fp32 storage, relaxed-precision PE path: 1
cycle/row at N=512 vs 4 for strict fp32; ~3e-4 rel err end to end).

To hide the input-DMA ramp (xT is 8 MB, W1 8 MB; ~350 GB/s effective), the
expert L1 loop is restructured: experts run in groups of 3 with the d-chunk
loop OUTER, so each arriving xT chunk immediately feeds 6 accumulating
matmuls (3 experts x 2 b-tiles) plus the gate matmuls in group 0 — the PE
ramps at DMA rate instead of stalling ~22 us. Weight DMAs go on a second
HWDGE queue (nc.scalar) so their issue doesn't serialize behind xT/out on
nc.sync. Output is written transposed ([D, Bs]) so every DMA is contiguous;
the host transposes it back.
"""

import sys

sys.path.insert(0, "/opt/trn_rl_repo")

import numpy as np

import concourse.bass as bass
import concourse.tile as tile
from concourse import bacc, mybir
from concourse import bass_utils

B, D, E, H = 8192, 2048, 8, 128
NCORES = 8
Bs = B // NCORES          # batch rows per core
BT = 512                  # moving-operand (free dim) tile
NBT = Bs // BT            # b-tiles per core
DCH = D // 128            # 128-row chunks of the D axis
GROUPS = [[0, 1, 2], [3, 4, 5], [6, 7]]
XT_EVERY_REP = True

F32 = mybir.dt.float32
F32R = mybir.dt.float32r
Silu = mybir.ActivationFunctionType.Silu
Exp = mybir.ActivationFunctionType.Exp


def _build_module(reps=1):
    nc = bacc.Bacc("TRN2", target_bir_lowering=False, debug=False,
                   num_devices=NCORES)

    xT = nc.dram_tensor("xT", [D, Bs], F32R, kind="ExternalInput").ap()
    gwr = nc.dram_tensor("gwr", [128, DCH, E], F32R, kind="ExternalInput").ap()
    gb = nc.dram_tensor("gb", [E, 1], F32, kind="ExternalInput").ap()
    w1r = nc.dram_tensor("w1r", [E, 128, DCH * H], F32R, kind="ExternalInput").ap()
    b1t = nc.dram_tensor("b1t", [H, E], F32, kind="ExternalInput").ap()
    w2 = nc.dram_tensor("w2", [E, H, H], F32R, kind="ExternalInput").ap()
    b2t = nc.dram_tensor("b2t", [H, E], F32, kind="ExternalInput").ap()
    w3r = nc.dram_tensor("w3r", [DCH, 128, E * H], F32R, kind="ExternalInput").ap()
    # oh[k, e*128 + p] = 1.0 iff k == e; lhsT slice e broadcasts gate row e
    # across 128 psum partitions via a K=8 matmul.
    oh = nc.dram_tensor("oh", [E, E * 128], F32R, kind="ExternalInput").ap()
    onesd = nc.dram_tensor("onesd", [E, E], F32R, kind="ExternalInput").ap()
    outT = nc.dram_tensor("outT", [D, Bs], F32, kind="ExternalOutput").ap()

    with tile.TileContext(nc) as tc:
        with (
            tc.tile_pool(name="persist", bufs=1) as persist,
            tc.tile_pool(name="stream", bufs=2) as stream,
        ):
            # gate weights first on the scalar queue: the very first PE
            # work (gate matmul on xT chunk 0) needs only gw
            gw_sb = persist.tile([128, DCH, E], F32R, tag="gw")
            nc.scalar.dma_start(gw_sb[:], gwr[:])
            # small constants: allocated now, DMA'd later (after the W1 head
            # stream) so they don't delay the first L1 matmuls
            ones8 = persist.tile([E, 1], F32R, tag="ones8")
            ones1x8 = persist.tile([1, E], F32R, tag="ones1x8")
            oh_sb = persist.tile([E, E * 128], F32R, tag="oh")
            gb_sb = persist.tile([E, 1], F32, tag="gb")
            b1_sb = persist.tile([H, E], F32, tag="b1")
            b2_sb = persist.tile([H, E], F32, tag="b2")
            w2_sb = persist.tile([H, E, H], F32R, tag="w2")

            def _load_smalls():
                nc.scalar.dma_start(ones8[:], onesd[:, 0:1])
                nc.scalar.dma_start(ones1x8[:], onesd[0:1, :])
                nc.scalar.dma_start(oh_sb[:], oh[:])
                nc.scalar.dma_start(gb_sb[:], gb[:])
                nc.scalar.dma_start(b1_sb[:], b1t[:])
                nc.scalar.dma_start(b2_sb[:], b2t[:])
                nc.scalar.dma_start(w2_sb[:], w2.rearrange("e h k -> h e k"))

            xt_tiles = [persist.tile([128, Bs], F32R, tag="xT", bufs=DCH,
                                     name=f"xt{dc}") for dc in range(DCH)]
            gn_sb = persist.tile([E, Bs], F32R, tag="gn")
            h2s_tiles = [persist.tile([128, Bs], F32R, tag="h2s", bufs=E,
                                      name=f"h2s{e}") for e in range(E)]

            for _rep in range(reps):
                with tc.tile_pool(name="psumA", bufs=1, space="PSUM") as psA:
                    pgate = [psA.tile([E, BT], F32, tag="gate", bufs=2,
                                      name=f"pg{bt}") for bt in range(NBT)]
                    w1_tiles = {}
                    h1_tiles = {}

                    def _w1_load(e, halves=False):
                        w1_tiles[e] = stream.tile([128, DCH, H], F32R,
                                                  tag="w1", bufs=5,
                                                  name=f"w1_{e}")
                        src = w1r[e].rearrange("p (dc h) -> p dc h", h=H)
                        hdc = DCH // 2
                        if halves:
                            nc.scalar.dma_start(w1_tiles[e][:, :hdc, :],
                                                src[:, :hdc, :])
                            nc.scalar.dma_start(w1_tiles[e][:, hdc:, :],
                                                src[:, hdc:, :])
                        else:
                            nc.scalar.dma_start(w1_tiles[e][:], src)

                    for gi, grp in enumerate(GROUPS):
                        if gi == 0:
                            _w1_load(grp[0], halves=True)

                        ph1 = {}
                        for e in grp:
                            for bt in range(NBT):
                                ph1[e, bt] = psA.tile([H, BT], F32, tag="acc",
                                                      bufs=6, name=f"ph1_{e}_{bt}")

                        # per-expert chunk issue order: in group 0, experts
                        # e1/e2 start late (their W1 DMA is staggered so xT
                        # keeps streaming) and catch up on the early chunks
                        # after the loop. start/stop flags follow issue order.
                        if gi == 0:
                            delay = {e: 4 * i for i, e in enumerate(grp)}
                        else:
                            delay = {e: 0 for e in grp}
                        sched = {e: list(range(d, DCH)) + list(range(d))
                                 for e, d in delay.items()}

                        nxt = GROUPS[gi + 1] if gi + 1 < len(GROUPS) else []
                        prefetch_at = {}
                        if gi == 0:
                            prefetch_at = {4 * i - 1: e
                                           for i, e in enumerate(grp) if i}
                            for i, e in enumerate(nxt):
                                prefetch_at[10 + 2 * i] = e
                        else:
                            for i, e in enumerate(nxt):
                                prefetch_at[2 + 2 * i] = e

                        for step in range(DCH):
                            dc = step
                            if gi == 0 and (_rep == 0 or XT_EVERY_REP):
                                src = xT[dc * 128:(dc + 1) * 128, :]
                                if dc == 0:
                                    nc.sync.dma_start(xt_tiles[dc][:, :BT],
                                                      src[:, :BT])
                                    nc.sync.dma_start(xt_tiles[dc][:, BT:],
                                                      src[:, BT:])
                                else:
                                    nc.sync.dma_start(xt_tiles[dc][:], src)
                            if step in prefetch_at:
                                _w1_load(prefetch_at[step])
                            if gi == 0 and dc == 8:
                                _load_smalls()
                            if gi == 0:
                                for bt in range(NBT):
                                    nc.tensor.matmul(
                                        pgate[bt][:], gw_sb[:, dc, :],
                                        xt_tiles[dc][:, bass.ts(bt, BT)],
                                        start=(dc == 0), stop=(dc == DCH - 1))
                            for e in grp:
                                if step < delay[e]:
                                    continue
                                cdc = sched[e][step - delay[e]]
                                for bt in range(NBT):
                                    nc.tensor.matmul(
                                        ph1[e, bt][:], w1_tiles[e][:, cdc, :],
                                        xt_tiles[cdc][:, bass.ts(bt, BT)],
                                        start=(step == delay[e]),
                                        stop=(step == DCH - 1 and delay[e] == 0))

                        if gi == 0:
                            # catch-up: chunks skipped while e1/e2 W1 was in
                            # flight (same accumulation groups, so no start)
                            for e in grp[1:]:
                                d = delay[e]
                                for j, cdc in enumerate(sched[e][DCH - d:]):
                                    for bt in range(NBT):
                                        nc.tensor.matmul(
                                            ph1[e, bt][:], w1_tiles[e][:, cdc, :],
                                            xt_tiles[cdc][:, bass.ts(bt, BT)],
                                            start=False, stop=(j == d - 1))
                            # softmax epilogue: gn[e, b] = exp(z+gb)/sum_e
                            for bt in range(NBT):
                                bs = bass.ts(bt, BT)
                                expT = stream.tile([E, BT], F32R, tag="expT")
                                nc.scalar.activation(expT[:], pgate[bt][:], Exp,
                                                     bias=gb_sb[:], scale=1.0)
                                psum_z = psA.tile([1, BT], F32, tag="gate", bufs=2)
                                nc.tensor.matmul(psum_z[:], ones8[:], expT[:],
                                                 start=True, stop=True)
                                recip = stream.tile([1, BT], F32R, tag="recip")
                                with nc.allow_low_precision(
                                        reason="f32r rounding of softmax denom"):
                                    nc.vector.reciprocal(recip[:], psum_z[:])
                                pr8 = psA.tile([E, BT], F32, tag="gate", bufs=2)
                                nc.tensor.matmul(pr8[:], ones1x8[:], recip[:],
                                                 start=True, stop=True)
                                nc.vector.tensor_mul(gn_sb[:, bs], expT[:],
                                                     pr8[:])

                        if gi == len(GROUPS) - 1:
                            w3_pre = []
                            for pdc in range(3):
                                w3p = stream.tile([128, E, H], F32R, tag="w3",
                                                  bufs=4, name=f"w3p{pdc}")
                                nc.scalar.dma_start(w3p[:], w3r[pdc].rearrange(
                                    "p (e h) -> p e h", e=E))
                                w3_pre.append(w3p)

                        # L2 + gating, batched by stage across the group so
                        # PE work stays contiguous instead of idling behind
                        # each expert's ACT chain
                        for e in grp:
                            h1_tiles[e] = stream.tile([H, Bs], F32R, tag="h1",
                                                      bufs=4, name=f"h1_{e}")
                            for bt in range(NBT):
                                bs = bass.ts(bt, BT)
                                nc.scalar.activation(h1_tiles[e][:, bs],
                                                     ph1[e, bt][:], Silu,
                                                     bias=b1_sb[:, e:e + 1],
                                                     scale=1.0)
                        ph2 = {}
                        for e in grp:
                            for bt in range(NBT):
                                bs = bass.ts(bt, BT)
                                ph2[e, bt] = psA.tile([H, BT], F32, tag="acc",
                                                      bufs=6, name=f"ph2_{e}_{bt}")
                                nc.tensor.matmul(ph2[e, bt][:], w2_sb[:, e, :],
                                                 h1_tiles[e][:, bs],
                                                 start=True, stop=True)
                        h2t = {}
                        for e in grp:
                            for bt in range(NBT):
                                h2t[e, bt] = stream.tile([H, BT], F32,
                                                         tag="h2t", bufs=6,
                                                         name=f"h2t_{e}_{bt}")
                                nc.scalar.activation(h2t[e, bt][:],
                                                     ph2[e, bt][:], Silu,
                                                     bias=b2_sb[:, e:e + 1],
                                                     scale=1.0)
                        pgb = {}
                        for e in grp:
                            for bt in range(NBT):
                                bs = bass.ts(bt, BT)
                                pgb[e, bt] = psA.tile([128, BT], F32, tag="acc",
                                                      bufs=6, name=f"pgb_{e}_{bt}")
                                nc.tensor.matmul(pgb[e, bt][:],
                                                 oh_sb[:, e * 128:(e + 1) * 128],
                                                 gn_sb[:, bs],
                                                 start=True, stop=True)
                        for e in grp:
                            for bt in range(NBT):
                                bs = bass.ts(bt, BT)
                                nc.vector.tensor_mul(h2s_tiles[e][:, bs],
                                                     h2t[e, bt][:], pgb[e, bt][:])

                # ---- output phase: outT[dc] = sum_e W3[e,dc].T @ h2s[e] ----
                with tc.tile_pool(name="psumB", bufs=1, space="PSUM") as psB:
                    for dc in range(DCH):
                        if dc < 3:
                            w3_sb = w3_pre[dc]
                        else:
                            w3_sb = stream.tile([128, E, H], F32R, tag="w3",
                                                bufs=4)
                            nc.scalar.dma_start(w3_sb[:], w3r[dc].rearrange(
                                "p (e h) -> p e h", e=E))
                        for bt in range(NBT):
                            bs = bass.ts(bt, BT)
                            po = psB.tile([128, BT], F32, tag="out", bufs=4)
                            for e in range(E):
                                nc.tensor.matmul(po[:], w3_sb[:, e, :],
                                                 h2s_tiles[e][:, bs],
                                                 start=(e == 0), stop=(e == E - 1))
                            o_sb = stream.tile([128, BT], F32, tag="osb", bufs=3)
                            nc.vector.tensor_copy(o_sb[:], po[:])
                            nc.sync.dma_start(outT[dc * 128:(dc + 1) * 128, bs],
                                              o_sb[:])

    nc.compile()
    return nc


_MODULE_CACHE = {}


def _get_module(reps=1):
    if reps not in _MODULE_CACHE:
        _MODULE_CACHE[reps] = _build_module(reps)
    return _MODULE_CACHE[reps]


def _prep_in_maps(x, gate_w, gate_b, W1, b1, W2, b2, W3):
    gwr = np.ascontiguousarray(
        gate_w.reshape(DCH, 128, E).transpose(1, 0, 2))
    gb = np.ascontiguousarray(gate_b.reshape(E, 1))
    w1r = np.ascontiguousarray(
        W1.reshape(E, DCH, 128, H).transpose(0, 2, 1, 3)).reshape(E, 128, DCH * H)
    w3r = np.ascontiguousarray(
        W3.reshape(E, H, DCH, 128).transpose(2, 1, 0, 3)).reshape(DCH, 128, E * H)

    oh = np.zeros((E, E * 128), dtype=np.float32)
    for e in range(E):
        oh[e, e * 128:(e + 1) * 128] = 1.0
    shared = {"gwr": gwr, "gb": gb, "w1r": w1r,
              "b1t": np.ascontiguousarray(b1.T),
              "w2": np.ascontiguousarray(W2),
              "b2t": np.ascontiguousarray(b2.T),
              "w3r": w3r, "oh": oh,
              "onesd": np.ones((E, E), dtype=np.float32)}
    in_maps = []
    for i in range(NCORES):
        xi = np.ascontiguousarray(x[i * Bs:(i + 1) * Bs, :].T)
        in_maps.append({"xT": xi, **shared})
    return in_maps


def kernel(x, gate_w, gate_b, W1, b1, W2, b2, W3, b3):
    x = np.asarray(x, dtype=np.float32)
    gate_w = np.asarray(gate_w, dtype=np.float32)
    gate_b = np.asarray(gate_b, dtype=np.float32)
    W1 = np.asarray(W1, dtype=np.float32)
    b1 = np.asarray(b1, dtype=np.float32)
    W2 = np.asarray(W2, dtype=np.float32)
    b2 = np.asarray(b2, dtype=np.float32)
    W3 = np.asarray(W3, dtype=np.float32)
    b3 = np.asarray(b3, dtype=np.float32)

    nc = _get_module(1)
    in_maps = _prep_in_maps(x, gate_w, gate_b, W1, b1, W2, b2, W3)
    try:
        res = bass_utils.run_bass_kernel_spmd(
            nc, in_maps, core_ids=list(range(NCORES)))
    except Exception:
        # the axon-tunneled devices occasionally report a transient
        # NRT_EXEC_UNIT_UNRECOVERABLE; one retry after a pause clears it
        import time as _time
        _time.sleep(30)
        res = bass_utils.run_bass_kernel_spmd(
            nc, in_maps, core_ids=list(range(NCORES)))

    out = np.empty((B, D), dtype=np.float32)
    for i in range(NCORES):
        out[i * Bs:(i + 1) * Bs, :] = res.results[i]["outT"].T

    if np.any(b3):
        # b3 contributes sum_e gates[b,e] * b3[e,d]; the device kernel skips
        # it (it is zero for this problem's inputs), so patch on host.
        logits = x @ gate_w + gate_b
        m = logits.max(axis=1, keepdims=True)
        p = np.exp(logits - m)
        gates = p / p.sum(axis=1, keepdims=True)
        out += gates @ b3
    return out

